# revision 3
# baseline (speedup 1.0000x reference)
"""Trainium2 Bass kernel for nn_GapDecoder.

Computes gaps[i,j] = proj[i] + proj[j] + b2 where
proj = relu(x @ W1 + b1) @ w2, x: [8192, 512] f32.

Strategy (8 NeuronCores, block-partitioned, collective-free):
  gaps is symmetric, so the device only stores ~half the [8192, 8192]
  output; the host places every stored tile at both its position and
  its transpose during the gather/upcast pass.  The [8192, 8192] array
  is an 8x8 grid of [1024, 1024] chunk blocks.  Core m reads x chunks
  {m, m+1, m+2, m+4} (a perfect difference set: every chunk-pair
  distance 1..4 appears inside it) and stores:
    - the upper staircase of diag block (m, m)      (36 of 64 microtiles)
    - full blocks (m, m+1), (m, m+2), (m+1, m+4)    (distances 1, 2, 3)
    - the upper staircase of block (m, m+4); its mirror half is the
      staircase stored by core m+4, whose block (m+4, m+8=m) transposes
      onto the lower microtiles.
  The union over cores covers every symmetric pair exactly (the d=4 and
  diag microtile diagonals overlap with identical values).  Every core
  emits the identical tile-shape sequence, so the single SPMD program
  is valid; only the host-side placement table differs per core.

  All HBM traffic and matmul operands are bf16 (rel-err budget 2e-2;
  bf16 costs ~0.5%): 4MB x read + 8.65MB output write per core.  DMA
  dispatch (DMA_DIRECT2D) costs ~0.6us per dma_start regardless of
  size, so each emission wave accumulates into one wide SBUF tile and
  ships as two large DMAs (10 store dispatches total).  The outer-sum
  adds are spread across DVE (tensor_scalar, 16-bit 2x), ACT
  (activation Identity with per-partition bias), and GpSimd so no
  single engine gates the DMA drain.  Reads dispatch per-stripe on the
  sync queue; stores go to the ACT queue early (sync still drains
  reads), then alternate.  Output is upcast to f32 + mirrored on host.
"""

import sys

sys.path.insert(0, "/opt/trn_rl_repo")

import ml_dtypes
import numpy as np

N, D, H = 8192, 512, 32
NCORES = 8
CHUNK = 1024  # block edge / proj chunk
NLOC = 4  # chunks per core
STRIPE = 512  # rows per PE stripe
KCH = D // 128
HALF = CHUNK // STRIPE  # stripes per chunk

# local chunk offsets (positions 0..3 hold chunks m, m+1, m+2, m+4)
LOCAL_OFFS = (0, 1, 2, 4)

# emission waves, one per local chunk position becoming ready.
# Each wave is one SBUF tile of 8 row-group adds; segment tuple is
# (col_in_wave, width, row_pos, row_group, col_pos, col_off).
# row_pos/col_pos index LOCAL_OFFS.
WAVES = []
for wv in range(5):
    segs = []
    c = 0
    for k in range(8):
        if wv == 0:  # diag staircase (m, m)
            w, rp, cp, c0 = CHUNK - 128 * k, 0, 0, 128 * k
        elif wv == 1:  # full block (m, m+1)
            w, rp, cp, c0 = CHUNK, 0, 1, 0
        elif wv == 2:  # full block (m, m+2)
            w, rp, cp, c0 = CHUNK, 0, 2, 0
        elif wv == 3:  # full block (m+1, m+4)
            w, rp, cp, c0 = CHUNK, 1, 3, 0
        else:  # staircase of (m, m+4)
            w, rp, cp, c0 = CHUNK - 128 * k, 0, 3, 128 * k
        segs.append((c, w, rp, k, cp, c0))
        c += w
    WAVES.append((segs, c))
TOTW = sum(w for _, w in WAVES)  # 33792
# add-engine pattern within each wave (V=DVE, A=ACT, G=GpSimd)
ENG_PAT = ("V", "A", "G", "V", "V", "A", "G", "V")
# store-dispatch engine per (wave, half): ACT queue while reads drain
# on sync, then balance bytes across both HW-DGE queues
DISP_PAT = {
    (0, 0): "A", (0, 1): "A",
    (1, 0): "A", (1, 1): "A",
    (2, 0): "S", (2, 1): "A",
    (3, 0): "S", (3, 1): "A",
    (4, 0): "S", (4, 1): "A",
}

# const blob layout (free offsets in the [128, *] tiles)
CB_W1 = 0  # bf16 [128, KCH*H]   w1, p-major
CB_W2B = KCH * H  # bf16 [32, 128]  w2 replicated (rows 0..31)
CB_W2 = KCH * H + 128  # bf16 [32, 1]
CB_BF_W = KCH * H + 129
CF_B1 = 0  # f32 [32, 1]
CF_B2 = 1  # f32 [128, 1]
CF_W = 2

_state = {}

# Set by run for test harnesses that want profile info (see test.py).
LAST_RESULTS = None


def _build():
    from concourse import bacc, tile, mybir

    f32 = mybir.dt.float32
    bf16 = mybir.dt.bfloat16
    nc = bacc.Bacc(
        "TRN2", target_bir_lowering=False, debug=False, num_devices=NCORES
    )

    # partition-major pack: chunk position L at rows [L*128, (L+1)*128),
    # row p = partition p holding [stripe, k, 512] contiguously
    xT_d = nc.dram_tensor(
        "xT4", [NLOC * 128, HALF * KCH * STRIPE], bf16, kind="ExternalInput"
    )
    cb_d = nc.dram_tensor("cb", [128, CB_BF_W], bf16, kind="ExternalInput")
    cf_d = nc.dram_tensor("cf", [128, CF_W], f32, kind="ExternalInput")
    out_d = nc.dram_tensor("out", [128, TOTW], bf16, kind="ExternalOutput")

    with tile.TileContext(nc) as tc:
        with (
            tc.tile_pool(name="const", bufs=1) as cpool,
            tc.tile_pool(name="xkp", bufs=4) as xkpool,
            tc.tile_pool(name="work", bufs=4) as wpool,
            tc.tile_pool(name="wave", bufs=5) as wavepool,
            tc.tile_pool(name="psum", bufs=4, space="PSUM") as pspool,
            tc.tile_pool(name="pspc", bufs=2, space="PSUM") as pspc,
            tc.tile_pool(name="psbc", bufs=2, space="PSUM") as psbc,
        ):
            # ---- constants: two packed blobs, two dispatches ----
            cb_sb = cpool.tile([128, CB_BF_W], bf16)
            nc.sync.dma_start(cb_sb[:], cb_d.ap())
            cf_sb = cpool.tile([128, CF_W], f32)
            nc.sync.dma_start(cf_sb[:], cf_d.ap())

            w1_sb = cb_sb[:, CB_W1 : CB_W1 + KCH * H].rearrange(
                "p (k h) -> p k h", k=KCH
            )
            w2b_sb = cb_sb[0:H, CB_W2B : CB_W2B + 128]
            w2_sb = cb_sb[0:H, CB_W2 : CB_W2 + 1]
            b1_sb = cf_sb[0:H, CF_B1 : CF_B1 + 1]
            b2b_sb = cf_sb[:, CF_B2 : CF_B2 + 1]

            # column proj (+b2) for all 4 chunk positions
            bcolall = cpool.tile([128, NLOC * CHUNK], bf16)
            # per-partition row-proj scalars for positions 0 and 1
            # (must stay f32: tensor_scalar requires an f32 scalar operand)
            projrow = [
                cpool.tile([128, CHUNK // 128], f32, name=f"projrow{i}")
                for i in range(2)
            ]

            # per-stripe x reads so the PE starts ~1us after the first
            # half-chunk lands instead of waiting for a whole chunk
            xks = []
            for loc in range(NLOC):
                xk = xkpool.tile(
                    [128, HALF, KCH * STRIPE], bf16, tag="xk", name=f"xk{loc}"
                )
                src = xT_d.ap()[loc * 128 : (loc + 1) * 128, :].rearrange(
                    "p (s k) -> p s k", s=HALF
                )
                for s in range(HALF):
                    nc.sync.dma_start(xk[:, s, :], src[:, s, :])
                xks.append(xk)

            def emit_wave(wv):
                segs, wavew = WAVES[wv]
                wt = wavepool.tile([128, wavew], bf16, tag="wt", name=f"wt{wv}")
                half_end = {0: 4, 1: 8}
                done = 0
                for i, (c, w, rp, g, cp, c0) in enumerate(segs):
                    src = bcolall[:, cp * CHUNK + c0 : cp * CHUNK + c0 + w]
                    dst = wt[:, c : c + w]
                    pr = projrow[rp][:, g : g + 1]
                    e = ENG_PAT[i]
                    if e == "V":
                        nc.vector.tensor_scalar_add(dst, src, pr)
                    elif e == "G":
                        nc.gpsimd.tensor_scalar_add(dst, src, pr)
                    else:
                        nc.scalar.activation(
                            dst, src, mybir.ActivationFunctionType.Identity,
                            bias=pr,
                        )
                    if i + 1 in (4, 8):
                        h = 0 if i + 1 == 4 else 1
                        cend = c + w
                        deng = nc.scalar if DISP_PAT[(wv, h)] == "A" else nc.sync
                        woff = sum(ww for _, ww in WAVES[:wv])
                        deng.dma_start(
                            out_d.ap()[:, woff + done : woff + cend],
                            wt[:, done:cend],
                        )
                        done = cend

            for loc in range(NLOC):
                xk = xks[loc]
                pc_ps = None
                if loc < 2:
                    # batched row-proj scalars: 4 one-column matmuls per
                    # stripe into one PSUM tile, one copy per stripe
                    pc_ps = pspc.tile([128, CHUNK // 128], f32, tag="pc")
                for half in range(HALF):
                    seqT_ps = pspool.tile([H, STRIPE], f32, tag="seqT")
                    for k in range(KCH):
                        nc.tensor.matmul(
                            seqT_ps[:],
                            w1_sb[:, k, :],
                            xk[:, half, k * STRIPE : (k + 1) * STRIPE],
                            start=(k == 0),
                            stop=(k == KCH - 1),
                        )
                    seqT_sb = wpool.tile([H, STRIPE], bf16, tag="seqT_sb")
                    # relu(x + b1) as a fused DVE op
                    nc.vector.tensor_scalar(
                        seqT_sb[:],
                        seqT_ps[:],
                        b1_sb,
                        0.0,
                        op0=mybir.AluOpType.add,
                        op1=mybir.AluOpType.max,
                    )
                    # broadcast proj of this stripe across all 128
                    # partitions in one matmul; ACT folds b2 into the
                    # psum->sbuf copy (Identity = in*1 + bias)
                    bc_ps = psbc.tile([128, STRIPE], f32, tag="bc")
                    nc.tensor.matmul(bc_ps[:], w2b_sb, seqT_sb[:])
                    base = loc * CHUNK
                    nc.scalar.activation(
                        bcolall[:, base + half * STRIPE : base + (half + 1) * STRIPE],
                        bc_ps[:],
                        mybir.ActivationFunctionType.Identity,
                        bias=b2b_sb,
                    )
                    # positions 2 and 3 never appear as band rows
                    if loc < 2:
                        npc = STRIPE // 128
                        for c in range(npc):
                            col = half * npc + c
                            nc.tensor.matmul(
                                pc_ps[:, col : col + 1],
                                seqT_sb[:, c * 128 : (c + 1) * 128],
                                w2_sb,
                            )
                        nc.vector.tensor_copy(
                            projrow[loc][:, half * npc : (half + 1) * npc],
                            pc_ps[:, half * npc : (half + 1) * npc],
                        )
                emit_wave(loc)
                if loc == 3:
                    emit_wave(4)

    nc.compile()
    return nc


def kernel(gathered_sequences, W1, b1, w2, b2):
    global LAST_RESULTS
    from concourse import bass_utils

    if "nc" not in _state:
        _state["nc"] = _build()
    nc = _state["nc"]

    bf = ml_dtypes.bfloat16
    x = np.ascontiguousarray(gathered_sequences, dtype=np.float32)
    xT = np.ascontiguousarray(x.T)  # [D, N]

    # const blobs
    cb = np.zeros((128, CB_BF_W), dtype=bf)
    W1b = np.asarray(W1, dtype=np.float32).astype(bf)  # [D, H]
    # w1 field: [p, k*H + h] = W1[k*128 + p, h]
    cb[:, CB_W1 : CB_W1 + KCH * H] = (
        W1b.reshape(KCH, 128, H).transpose(1, 0, 2).reshape(128, KCH * H)
    )
    w2c = np.reshape(w2, (H, 1)).astype(np.float32).astype(bf)
    cb[0:H, CB_W2B : CB_W2B + 128] = np.repeat(w2c, 128, axis=1)
    cb[0:H, CB_W2 : CB_W2 + 1] = w2c
    cf = np.zeros((128, CF_W), dtype=np.float32)
    cf[0:H, CF_B1] = np.reshape(b1, (H,)).astype(np.float32)
    cf[:, CF_B2] = float(np.reshape(b2, ()))

    in_maps = []
    for m in range(NCORES):
        locs = [(m + a) % NCORES for a in LOCAL_OFFS]
        xT4 = np.concatenate(
            [xT[:, L * CHUNK : (L + 1) * CHUNK] for L in locs], axis=1
        )  # [D, NLOC*CHUNK]
        # partition-major pack: [NLOC*128, HALF*KCH*STRIPE] where row
        # L*128+p holds position L's [stripe, k, 512] block for partition p
        xT4p = np.ascontiguousarray(
            xT4.reshape(KCH, 128, NLOC, HALF, STRIPE)
            .transpose(2, 1, 3, 0, 4)
            .reshape(NLOC * 128, HALF * KCH * STRIPE)
            .astype(bf)
        )
        in_maps.append({"xT4": xT4p, "cb": cb, "cf": cf})

    res = bass_utils.run_bass_kernel_spmd(nc, in_maps, core_ids=list(range(NCORES)))
    LAST_RESULTS = res

    out = np.empty((N, N), dtype=np.float32)
    for m in range(NCORES):
        locs = [(m + a) % NCORES for a in LOCAL_OFFS]
        buf = np.asarray(res.results[m]["out"]).astype(np.float32)
        woff = 0
        for segs, wavew in WAVES:
            for c, w, rp, g, cp, c0 in segs:
                gr = locs[rp] * CHUNK + g * 128
                gc = locs[cp] * CHUNK + c0
                blk = buf[:, woff + c : woff + c + w]
                out[gr : gr + 128, gc : gc + w] = blk
                out[gc : gc + w, gr : gr + 128] = blk.T
            woff += wavew
    return out


# revision 5
# speedup vs baseline: 2.8750x; 2.8750x over previous
"""Trainium2 Bass kernel for nn_GapDecoder.

Computes gaps[i,j] = proj[i] + proj[j] + b2 where
proj = relu(x @ W1 + b1) @ w2, x: [8192, 512] f32.

Strategy (8 NeuronCores, block-partitioned, collective-free):
  gaps is symmetric, so the device only stores ~half the [8192, 8192]
  output; the host places every stored tile at both its position and
  its transpose during the gather/upcast pass.  The [8192, 8192] array
  is an 8x8 grid of [1024, 1024] chunk blocks.  Core m reads x chunks
  {m, m+1, m+2, m+4} (a perfect difference set: every chunk-pair
  distance 1..4 appears inside it) and stores:
    - the upper staircase of diag block (m, m)      (36 of 64 microtiles)
    - full blocks (m, m+1), (m, m+2), (m+1, m+4)    (distances 1, 2, 3)
    - the upper staircase of block (m, m+4); its mirror half is the
      staircase stored by core m+4, whose block (m+4, m+8=m) transposes
      onto the lower microtiles.
  The union over cores covers every symmetric pair exactly (the d=4 and
  diag microtile diagonals overlap with identical values).  Every core
  emits the identical tile-shape sequence, so the single SPMD program
  is valid; only the host-side placement table differs per core.

  All HBM traffic and matmul operands are bf16 (rel-err budget 2e-2;
  bf16 costs ~0.5%): 4MB x read + 8.65MB output write per core.  DMA
  dispatch (DMA_DIRECT2D) costs ~0.6us per dma_start regardless of
  size, so each emission wave accumulates into one wide SBUF tile and
  ships as two large DMAs (10 store dispatches total).  The outer-sum
  adds are spread across DVE (tensor_scalar, 16-bit 2x), ACT
  (activation Identity with per-partition bias), and GpSimd so no
  single engine gates the DMA drain.  Reads dispatch per-stripe on the
  sync queue; stores go to the ACT queue early (sync still drains
  reads), then alternate.  Output is upcast to f32 + mirrored on host.
"""

import sys

sys.path.insert(0, "/opt/trn_rl_repo")

import ml_dtypes
import numpy as np

N, D, H = 8192, 512, 32
NCORES = 8
CHUNK = 1024  # block edge / proj chunk
NLOC = 4  # chunks per core
STRIPE = 512  # rows per PE stripe
KCH = D // 128
HALF = CHUNK // STRIPE  # stripes per chunk

# local chunk offsets (positions 0..3 hold chunks m, m+1, m+2, m+4)
LOCAL_OFFS = (0, 1, 2, 4)

# emission waves, one per local chunk position becoming ready.
# Each wave is one SBUF tile of 8 row-group adds; segment tuple is
# (col_in_wave, width, row_pos, row_group, col_pos, col_off).
# row_pos/col_pos index LOCAL_OFFS.
WAVES = []
for wv in range(5):
    segs = []
    c = 0
    for k in range(8):
        if wv == 0:  # diag staircase (m, m)
            w, rp, cp, c0 = CHUNK - 128 * k, 0, 0, 128 * k
        elif wv == 1:  # full block (m, m+1)
            w, rp, cp, c0 = CHUNK, 0, 1, 0
        elif wv == 2:  # full block (m, m+2)
            w, rp, cp, c0 = CHUNK, 0, 2, 0
        elif wv == 3:  # full block (m+1, m+4)
            w, rp, cp, c0 = CHUNK, 1, 3, 0
        else:  # staircase of (m, m+4)
            w, rp, cp, c0 = CHUNK - 128 * k, 0, 3, 128 * k
        segs.append((c, w, rp, k, cp, c0))
        c += w
    WAVES.append((segs, c))
TOTW = sum(w for _, w in WAVES)  # 33792
# add-engine pattern within each wave (V=DVE, A=ACT; GpSimd's
# tensor_scalar is ~20x slower than DVE and stalls concurrent engines)
ENG_PAT = ("V", "A", "A", "V", "V", "V", "A", "V")
# store-dispatch engine per (wave, half): ACT queue while reads drain
# on sync, then balance bytes across both HW-DGE queues
DISP_PAT = {
    (0, 0): "A", (0, 1): "A",
    (1, 0): "A", (1, 1): "A",
    (2, 0): "S", (2, 1): "S",
    (3, 0): "S", (3, 1): "S",
    (4, 0): "S", (4, 1): "S",
}

# const blob layout (free offsets in the [128, *] tiles)
CB_W1 = 0  # bf16 [128, KCH*H]   w1, p-major
CB_W2B = KCH * H  # bf16 [32, 128]  w2 replicated (rows 0..31)
CB_W2 = KCH * H + 128  # bf16 [32, 1]
CB_BF_W = KCH * H + 129
CF_B1 = 0  # f32 [32, 1]
CF_B2 = 1  # f32 [128, 1]
CF_W = 2

_state = {}

# Set by run for test harnesses that want profile info (see test.py).
LAST_RESULTS = None


def _build():
    from concourse import bacc, tile, mybir

    f32 = mybir.dt.float32
    bf16 = mybir.dt.bfloat16
    nc = bacc.Bacc(
        "TRN2", target_bir_lowering=False, debug=False, num_devices=NCORES
    )

    # partition-major pack: chunk position L at rows [L*128, (L+1)*128),
    # row p = partition p holding [stripe, k, 512] contiguously
    xT_d = nc.dram_tensor(
        "xT4", [NLOC * 128, HALF * KCH * STRIPE], bf16, kind="ExternalInput"
    )
    cb_d = nc.dram_tensor("cb", [128, CB_BF_W], bf16, kind="ExternalInput")
    cf_d = nc.dram_tensor("cf", [128, CF_W], f32, kind="ExternalInput")
    out_d = nc.dram_tensor("out", [128, TOTW], bf16, kind="ExternalOutput")

    with tile.TileContext(nc) as tc:
        with (
            tc.tile_pool(name="const", bufs=1) as cpool,
            tc.tile_pool(name="xkp", bufs=4) as xkpool,
            tc.tile_pool(name="work", bufs=4) as wpool,
            tc.tile_pool(name="wave", bufs=5) as wavepool,
            tc.tile_pool(name="psum", bufs=4, space="PSUM") as pspool,
            tc.tile_pool(name="pspc", bufs=2, space="PSUM") as pspc,
            tc.tile_pool(name="psbc", bufs=2, space="PSUM") as psbc,
        ):
            # ---- constants: two packed blobs, two dispatches ----
            cb_sb = cpool.tile([128, CB_BF_W], bf16)
            nc.sync.dma_start(cb_sb[:], cb_d.ap())
            cf_sb = cpool.tile([128, CF_W], f32)
            nc.sync.dma_start(cf_sb[:], cf_d.ap())

            w1_sb = cb_sb[:, CB_W1 : CB_W1 + KCH * H].rearrange(
                "p (k h) -> p k h", k=KCH
            )
            w2b_sb = cb_sb[0:H, CB_W2B : CB_W2B + 128]
            w2_sb = cb_sb[0:H, CB_W2 : CB_W2 + 1]
            b1_sb = cf_sb[0:H, CF_B1 : CF_B1 + 1]
            b2b_sb = cf_sb[:, CF_B2 : CF_B2 + 1]

            # column proj (+b2) for all 4 chunk positions
            bcolall = cpool.tile([128, NLOC * CHUNK], bf16)
            # per-partition row-proj scalars for positions 0 and 1
            # (must stay f32: tensor_scalar requires an f32 scalar operand)
            projrow = [
                cpool.tile([128, CHUNK // 128], f32, name=f"projrow{i}")
                for i in range(2)
            ]

            # per-stripe x reads so the PE starts ~1us after the first
            # half-chunk lands instead of waiting for a whole chunk
            xks = []
            for loc in range(NLOC):
                xk = xkpool.tile(
                    [128, HALF, KCH * STRIPE], bf16, tag="xk", name=f"xk{loc}"
                )
                src = xT_d.ap()[loc * 128 : (loc + 1) * 128, :].rearrange(
                    "p (s k) -> p s k", s=HALF
                )
                for s in range(HALF):
                    nc.sync.dma_start(xk[:, s, :], src[:, s, :])
                xks.append(xk)

            def emit_wave(wv):
                segs, wavew = WAVES[wv]
                wt = wavepool.tile([128, wavew], bf16, tag="wt", name=f"wt{wv}")
                half_end = {0: 4, 1: 8}
                done = 0
                for i, (c, w, rp, g, cp, c0) in enumerate(segs):
                    src = bcolall[:, cp * CHUNK + c0 : cp * CHUNK + c0 + w]
                    dst = wt[:, c : c + w]
                    pr = projrow[rp][:, g : g + 1]
                    e = ENG_PAT[i]
                    if e == "V":
                        nc.vector.tensor_scalar_add(dst, src, pr)
                    else:
                        nc.scalar.activation(
                            dst, src, mybir.ActivationFunctionType.Identity,
                            bias=pr,
                        )
                    if i + 1 in (4, 8):
                        h = 0 if i + 1 == 4 else 1
                        cend = c + w
                        deng = nc.scalar if DISP_PAT[(wv, h)] == "A" else nc.sync
                        woff = sum(ww for _, ww in WAVES[:wv])
                        deng.dma_start(
                            out_d.ap()[:, woff + done : woff + cend],
                            wt[:, done:cend],
                        )
                        done = cend

            for loc in range(NLOC):
                xk = xks[loc]
                pc_ps = None
                if loc < 2:
                    # batched row-proj scalars: 4 one-column matmuls per
                    # stripe into one PSUM tile, one copy per stripe
                    pc_ps = pspc.tile([128, CHUNK // 128], f32, tag="pc")
                for half in range(HALF):
                    seqT_ps = pspool.tile([H, STRIPE], f32, tag="seqT")
                    for k in range(KCH):
                        nc.tensor.matmul(
                            seqT_ps[:],
                            w1_sb[:, k, :],
                            xk[:, half, k * STRIPE : (k + 1) * STRIPE],
                            start=(k == 0),
                            stop=(k == KCH - 1),
                        )
                    seqT_sb = wpool.tile([H, STRIPE], bf16, tag="seqT_sb")
                    # relu(x + b1) as a fused DVE op
                    nc.vector.tensor_scalar(
                        seqT_sb[:],
                        seqT_ps[:],
                        b1_sb,
                        0.0,
                        op0=mybir.AluOpType.add,
                        op1=mybir.AluOpType.max,
                    )
                    # broadcast proj of this stripe across all 128
                    # partitions in one matmul; ACT folds b2 into the
                    # psum->sbuf copy (Identity = in*1 + bias)
                    bc_ps = psbc.tile([128, STRIPE], f32, tag="bc")
                    nc.tensor.matmul(bc_ps[:], w2b_sb, seqT_sb[:])
                    base = loc * CHUNK
                    nc.scalar.activation(
                        bcolall[:, base + half * STRIPE : base + (half + 1) * STRIPE],
                        bc_ps[:],
                        mybir.ActivationFunctionType.Identity,
                        bias=b2b_sb,
                    )
                    # positions 2 and 3 never appear as band rows
                    if loc < 2:
                        npc = STRIPE // 128
                        for c in range(npc):
                            col = half * npc + c
                            nc.tensor.matmul(
                                pc_ps[:, col : col + 1],
                                seqT_sb[:, c * 128 : (c + 1) * 128],
                                w2_sb,
                            )
                        nc.vector.tensor_copy(
                            projrow[loc][:, half * npc : (half + 1) * npc],
                            pc_ps[:, half * npc : (half + 1) * npc],
                        )
                emit_wave(loc)
                if loc == 3:
                    emit_wave(4)

    nc.compile()
    return nc


def kernel(gathered_sequences, W1, b1, w2, b2):
    global LAST_RESULTS
    from concourse import bass_utils

    if "nc" not in _state:
        _state["nc"] = _build()
    nc = _state["nc"]

    bf = ml_dtypes.bfloat16
    x = np.ascontiguousarray(gathered_sequences, dtype=np.float32)
    xT = np.ascontiguousarray(x.T)  # [D, N]

    # const blobs
    cb = np.zeros((128, CB_BF_W), dtype=bf)
    W1b = np.asarray(W1, dtype=np.float32).astype(bf)  # [D, H]
    # w1 field: [p, k*H + h] = W1[k*128 + p, h]
    cb[:, CB_W1 : CB_W1 + KCH * H] = (
        W1b.reshape(KCH, 128, H).transpose(1, 0, 2).reshape(128, KCH * H)
    )
    w2c = np.reshape(w2, (H, 1)).astype(np.float32).astype(bf)
    cb[0:H, CB_W2B : CB_W2B + 128] = np.repeat(w2c, 128, axis=1)
    cb[0:H, CB_W2 : CB_W2 + 1] = w2c
    cf = np.zeros((128, CF_W), dtype=np.float32)
    cf[0:H, CF_B1] = np.reshape(b1, (H,)).astype(np.float32)
    cf[:, CF_B2] = float(np.reshape(b2, ()))

    in_maps = []
    for m in range(NCORES):
        locs = [(m + a) % NCORES for a in LOCAL_OFFS]
        xT4 = np.concatenate(
            [xT[:, L * CHUNK : (L + 1) * CHUNK] for L in locs], axis=1
        )  # [D, NLOC*CHUNK]
        # partition-major pack: [NLOC*128, HALF*KCH*STRIPE] where row
        # L*128+p holds position L's [stripe, k, 512] block for partition p
        xT4p = np.ascontiguousarray(
            xT4.reshape(KCH, 128, NLOC, HALF, STRIPE)
            .transpose(2, 1, 3, 0, 4)
            .reshape(NLOC * 128, HALF * KCH * STRIPE)
            .astype(bf)
        )
        in_maps.append({"xT4": xT4p, "cb": cb, "cf": cf})

    res = bass_utils.run_bass_kernel_spmd(nc, in_maps, core_ids=list(range(NCORES)))
    LAST_RESULTS = res

    out = np.empty((N, N), dtype=np.float32)
    for m in range(NCORES):
        locs = [(m + a) % NCORES for a in LOCAL_OFFS]
        buf = np.asarray(res.results[m]["out"]).astype(np.float32)
        woff = 0
        for segs, wavew in WAVES:
            for c, w, rp, g, cp, c0 in segs:
                gr = locs[rp] * CHUNK + g * 128
                gc = locs[cp] * CHUNK + c0
                blk = buf[:, woff + c : woff + c + w]
                out[gr : gr + 128, gc : gc + w] = blk
                out[gc : gc + w, gr : gr + 128] = blk.T
            woff += wavew
    return out
